# revision 92
# baseline (speedup 1.0000x reference)
"""Causal self-attention (B=2, T=2048, D=2048, H=16, hd=128, RoPE via
host-built tables) as a Bass/Tile kernel on 8 Trainium2 NeuronCores.

Sharding: core c handles batch b=c//4 and heads 4*(c%4)..4*(c%4)+3 (data
parallel on B x tensor parallel on H).  Each core computes a partial output
projection; the host sums the 4 partials per batch.

Mixed-precision strategy (validated against the fp64 reference at ~4e-3 L2):
  - QKV / output projections run as 3-term error-compensated fp8 matmuls in
    DoubleRow perf mode: A@B ~ A_hi@B_hi + A_lo@B_hi + A_hi@B_lo with hi/lo
    e4m3 splits of both operands.  Weights are pre-scaled by 64 on the host
    so both hi and lo land in e4m3's normal range; the inverse scales fold
    into the PSUM->SBUF copy scale, the exp() scale and the host-side
    partial sum, costing nothing.
  - Attention (scores / probabilities / PV) runs in fp16: q,k,v are stored
    at 4x true scale, scores sit in fp32 PSUM, exp() folds the rescale.
  - Causality is exploited at 128-column granularity: for k-block kb only
    q >= 128*kb is ever computed (scores, exp, PV, denominator).
  - Softmax runs without max-subtraction (scores are O(1)); the denominator
    comes from a (1/16)-valued ones matmul so the normalizing multiply needs
    no broadcast and lands outT at 64*O, exactly the range the fp8 hi/lo
    split of phase 3 wants.  Probabilities are pre-collapsed across kb-quads
    on DVE (plus two pair-sums for the diagonal quad), cutting the
    denominator's PE rows per head from 17408 to ~6656.
  - RoPE is a pure partition-pair swap (strided SBUF->SBUF DMA on otherwise
    idle queues) plus two DVE multiplies against sign-folded cos/sin tables
    -- no PE or PSUM involvement.
  - Scheduling: each quad's score/exp blocks are emitted a full q-block
    ahead of their first PV read; the next head's first eight blocks boot
    inside the previous head's last PV accumulation; phase-3 output groups
    drip between attention work; startup operands stream in consumption-order chunks sized against
    the HWDGE's ~625ns per-DMA issue serialization.
"""

import sys

sys.path.insert(0, "/opt/trn_rl_repo")

import numpy as np

import concourse.bass as bass
import concourse.mybir as mybir
import concourse.tile as tile
from concourse.bass_utils import run_bass_kernel_spmd

F32 = mybir.dt.float32
F16 = mybir.dt.float16
FP8 = mybir.dt.float8e4
DR = mybir.MatmulPerfMode.DoubleRow

B = 2
T = 2048
D = 2048
H = 16
HD = 128
N_CORES = 8
HPC = 4           # heads per core
CORES_PER_B = 4
P = 128
TB = 512
KO2 = D // (2 * P)  # 8 paired contraction subtiles
NTB = T // TB       # 4
NQK = 2 * HPC       # 8 q+k dout tiles of 128
NKB = T // P        # 16 k-blocks per head
SW = 64.0           # host weight pre-scale (fp8 range)
QS = 4.0            # q,k,v live in SBUF at QS x true scale
SO = 16.0           # outT holds SO*O; O is a convex mix of v so
                    # |SO*O| <= SO*max|v| ~ 80, inside e4m3 range
EXP_SCALE = 1.0 / (QS * QS * float(np.sqrt(HD)))
EXP_BIAS = -4.0     # global exp shift, cancels in the softmax ratio;
                    # keeps fp16 probabilities well under 65504


# ---------------------------------------------------------------------------
# Walrus on this toolchain rejects instructions carrying more than one sync
# wait command; Tile can emit several (e.g. the kernel-tail drain).  Hoist
# the excess onto injected same-engine NoOps — semantically identical.
def _fix_waits(nc, cap=1):
    ctr = 0
    for f in nc.m.functions:
        for bb in f.blocks:
            insts = bb.instructions
            i = 0
            while i < len(insts):
                inst = insts[i]
                si = inst.sync_info
                if si is not None and si.on_wait and len(si.on_wait) > cap:
                    waits = list(si.on_wait)
                    if len(waits) > 6:
                        # kernel-tail drain: DMA-completion sems are satisfied
                        # last, so waiting them first lets the remaining 50ns
                        # NoOp waits absorb into that single wait
                        waits.sort(
                            key=lambda w: 0 if "DMA" in (w.ant_name or "")
                            or "DGE" in (w.ant_name or "") else 1)
                    keep, excess = waits[:cap], waits[cap:]
                    nops = []
                    for j in range(0, len(excess), cap):
                        ctr += 1
                        nops.append(
                            mybir.InstNoOp(
                                name=f"I-waitfix-{ctr}",
                                engine=inst.engine,
                                sync_info=mybir.SyncInfo(
                                    on_wait=excess[j : j + cap], on_update=[]
                                ),
                            )
                        )
                    inst.sync_info = mybir.SyncInfo(
                        on_wait=keep, on_update=list(si.on_update or [])
                    )
                    insts[i:i] = nops
                    i += len(nops)
                i += 1
    return ctr


def _build_program():
    import os
    debug = bool(os.environ.get("KDBG"))
    nc = bass.Bass()

    xh8 = nc.dram_tensor("xh8", (D, T), FP8, kind="ExternalInput")
    xl8 = nc.dram_tensor("xl8", (D, T), FP8, kind="ExternalInput")
    wqkh = nc.dram_tensor("wqkh", (D, NQK * P), FP8, kind="ExternalInput")
    wqkl = nc.dram_tensor("wqkl", (D, NQK * P), FP8, kind="ExternalInput")
    wvh = nc.dram_tensor("wvh", (D, HPC * HD), FP8, kind="ExternalInput")
    wvl = nc.dram_tensor("wvl", (D, HPC * HD), FP8, kind="ExternalInput")
    woh = nc.dram_tensor("woh", (HPC * HD, D), FP8, kind="ExternalInput")
    wol = nc.dram_tensor("wol", (HPC * HD, D), FP8, kind="ExternalInput")
    ones = nc.dram_tensor("ones", (P, P), F16, kind="ExternalInput")
    cs = nc.dram_tensor("cs", (HPC, 2, P, T), F16, kind="ExternalInput")
    maskd = nc.dram_tensor("maskd", (P, P), F16, kind="ExternalInput")
    ebias = nc.dram_tensor("ebias", (P, 1), F32, kind="ExternalInput")
    y = nc.dram_tensor("y", (T, D), F16, kind="ExternalOutput")
    if debug:
        dqk = nc.dram_tensor("dqk", (NQK, P, T), F16, kind="ExternalOutput")
        dv = nc.dram_tensor("dv", (P, NKB, HPC * HD), F16,
                            kind="ExternalOutput")
        dpt = nc.dram_tensor("dpt", (P, NKB, T), F16, kind="ExternalOutput")
        doh = nc.dram_tensor("doh", (2, P, 2, T), FP8, kind="ExternalOutput")
        dol = nc.dram_tensor("dol", (2, P, 2, T), FP8, kind="ExternalOutput")
        drp = nc.dram_tensor("drp", (2, P, T), F16, kind="ExternalOutput")

    with tile.TileContext(nc) as tc:
        with (
            tc.tile_pool(name="consts", bufs=1) as consts,
            tc.tile_pool(name="qk", bufs=1) as qk_pool,
            tc.tile_pool(name="vsb", bufs=1) as v_pool,
            tc.tile_pool(name="o8", bufs=1) as o8_pool,
            tc.tile_pool(name="cs", bufs=2) as cs_pool,
            tc.tile_pool(name="rope", bufs=2) as rope_pool,
            tc.tile_pool(name="t16", bufs=2) as t16_pool,
            tc.tile_pool(name="rec", bufs=1) as rec_pool,
        ):
            # const tiles are allocated here but their DMAs are emitted after
            # phase 1's first operand loads, keeping the DMA queues clear for
            # the first matmul group's dependencies
            ones_sb = consts.tile([P, P], F16)
            maskd_sb = consts.tile([P, P], F16)
            ebias_sb = consts.tile([P, 1], F32)

            def load_consts():
                nc.sync.dma_start(ones_sb[:], ones[:])
                nc.sync.dma_start(maskd_sb[:], maskd[:])
                nc.sync.dma_start(ebias_sb[:], ebias[:])

            qk_sb = [qk_pool.tile([P, T], F16, tag=f"qk{d}", name=f"qk{d}")
                     for d in range(NQK)]
            v_sb = v_pool.tile([P, NKB, HPC * HD], F16)
            # fp8 hi/lo of 64*O, one tile per head pair, [hd, head, t]
            o_hi = [o8_pool.tile([P, 2, T], FP8, tag=f"ohi{a}", name=f"ohi{a}")
                    for a in range(2)]
            o_lo = [o8_pool.tile([P, 2, T], FP8, tag=f"olo{a}", name=f"olo{a}")
                    for a in range(2)]

            cs_tiles = []

            def load_cs(h):
                t = cs_pool.tile([P, 2, T], F16, tag="cs", name=f"cs{h}")
                nc.sync.dma_start(t[:], cs[h].rearrange("c p t -> p c t"))
                cs_tiles.append(t)

            def alloc_rope():
                qr = rope_pool.tile([P, T], F16, tag="qr")
                kr = rope_pool.tile([P, T], F16, tag="kr")
                return qr, kr

            def rope_block(h, which, tb):
                """One 512-wide block of RoPE for head h (which: 0=q 1=k):
                roped = C*t + S'*swap(t), where swap exchanges partition
                pairs (a pure DMA) and S' carries the rotation signs.  The
                swap lands in the destination tile, which the C*t multiply
                then overwrites after S'*u is consumed — no extra SBUF and
                no PE/PSUM involvement."""
                src_d = h + HPC * which
                dst = roped[h][which]
                cs_h = cs_tiles[h]
                sl = slice(tb * TB, (tb + 1) * TB)
                srct = qk_sb[src_d]
                # u = swap(t) into dst, one strided DMA per parity; during
                # phase 1 (head 0) these ride the software DGE so the HWDGE
                # queues stay clear for the input stream
                q1, q2 = ((nc.gpsimd, nc.gpsimd) if h == 0
                          else (nc.sync, nc.scalar))
                q1.dma_start(dst[0:P:2, sl], srct[1:P:2, sl])
                q2.dma_start(dst[1:P:2, sl], srct[0:P:2, sl])
                tmp = t16_pool.tile([P, TB], F16, tag="ropetmp",
                                    name=f"rtmp{h}_{src_d}_{tb}")
                nc.vector.tensor_tensor(tmp[:], dst[:, sl], cs_h[:, 1, sl],
                                        mybir.AluOpType.mult)
                nc.vector.tensor_tensor(dst[:, sl], srct[:, sl],
                                        cs_h[:, 0, sl], mybir.AluOpType.mult)
                nc.vector.tensor_tensor(dst[:, sl], dst[:, sl], tmp[:],
                                        mybir.AluOpType.add)

            roped = [alloc_rope()]

            # ----------------------------------------------------------- p1
            xh_r = xh8.rearrange("(a j p) t -> p a j t", j=2, p=P)
            xl_r = xl8.rearrange("(a j p) t -> p a j t", j=2, p=P)
            with (
                tc.tile_pool(name="p1w", bufs=1) as p1w,
                tc.tile_pool(name="p1x", bufs=2) as p1x,
                tc.tile_pool(name="p1p", bufs=8, space="PSUM") as p1p,
            ):
                # Weight loads are interleaved per ko-pair with the first
                # t-block's x loads (emitted in the tb loop below) so the
                # first matmul group's operands land within a few microseconds
                # instead of behind the whole weight stream.
                # Each dma_start costs its queue's sequencer ~1.2us of
                # descriptor generation, so operands are loaded as a few
                # monolithic transfers (split once for startup pacing), not
                # per-subtile.  xh rides the ACT hwdge queue, everything else
                # the SP queue.
                wqkh_sb = p1w.tile([P, KO2, 2, NQK * P], FP8, name="wqkh")
                wqkl_sb = p1w.tile([P, KO2, 2, NQK * P], FP8, name="wqkl")
                wvh_sb = p1w.tile([P, KO2, 2, HPC * HD], FP8, name="wvh")
                wvl_sb = p1w.tile([P, KO2, 2, HPC * HD], FP8, name="wvl")
                wqkh_r = wqkh.rearrange("(a j p) n -> p a j n", j=2, p=P)
                wqkl_r = wqkl.rearrange("(a j p) n -> p a j n", j=2, p=P)

                def load_x(tb, tsl):
                    th = p1x.tile([P, KO2, 2, TB], FP8, tag="xh",
                                  name=f"xh{tb}")
                    tl = p1x.tile([P, KO2, 2, TB], FP8, tag="xl",
                                  name=f"xl{tb}")
                    nc.scalar.dma_start(th[:], xh_r[:, :, :, tsl])
                    nc.sync.dma_start(tl[:], xl_r[:, :, :, tsl])
                    return th, tl

                # startup: the shared DMA device drains transfers roughly in
                # issue order, so issue them in the order phase 1's term
                # sweeps consume them: xh (ACT queue) || wqkh, wqkl, xl (SP).
                # The first term sweep walks a=0..7 with only (wqkh, xh)
                # operands, so those two are chunked per-a: the a=0 slices
                # (384 KB) land ~2us in and the PE starts immediately instead
                # of waiting out the full 3 MB.
                x0h = p1x.tile([P, KO2, 2, TB], FP8, tag="xh", name="xh0")
                x0l = p1x.tile([P, KO2, 2, TB], FP8, tag="xl", name="xl0")
                # PE p-state warmup: the tensor engine reaches full clock only
                # after ~3us of activity, and PE idles that long waiting for
                # the first operand DMAs anyway.  Junk matmuls on a zeroed
                # tile (into a ps1-tag PSUM generation that the first real
                # accumulation group resets via start=True) ride out the ramp
                # for free.
                nc.scalar.dma_start(x0h[:, 0], xh_r[:, 0, :, 0:TB])
                nc.sync.dma_start(wqkh_sb[:, 0, :, : NQK * P // 2],
                                  wqkh_r[:, 0, :, : NQK * P // 2])
                nc.sync.dma_start(wqkh_sb[:, 0, :, NQK * P // 2 :],
                                  wqkh_r[:, 0, :, NQK * P // 2 :])
                for a, b in ((1, 3), (3, 5), (5, 7), (7, 8)):
                    nc.scalar.dma_start(x0h[:, a:b], xh_r[:, a:b, :, 0:TB])
                for a in range(1, KO2):
                    nc.sync.dma_start(wqkh_sb[:, a], wqkh_r[:, a])
                nc.sync.dma_start(wqkl_sb[:, :3], wqkl_r[:, :3])
                nc.sync.dma_start(wqkl_sb[:, 3:6], wqkl_r[:, 3:6])
                nc.sync.dma_start(wqkl_sb[:, 6:], wqkl_r[:, 6:])
                nc.sync.dma_start(x0l[:], xl_r[:, :, :, 0:TB])
                x0 = (x0h, x0l)
                load_cs(0)
                nc.sync.dma_start(
                    wvh_sb[:], wvh.rearrange("(a j p) n -> p a j n", j=2, p=P))
                nc.sync.dma_start(
                    wvl_sb[:], wvl.rearrange("(a j p) n -> p a j n", j=2, p=P))
                load_consts()


                copy_rot = [0]

                def scaled_copy(dst, src):
                    # PSUM(fp32, 64x) -> SBUF(fp16, QSx): scale by QS/SW
                    r = copy_rot[0] % 2
                    copy_rot[0] += 1
                    if r == 0:
                        nc.vector.tensor_scalar_mul(dst, src, QS / SW)
                    else:
                        nc.scalar.mul(dst, src, QS / SW)

                for tb in range(NTB):
                    tsl = slice(tb * TB, (tb + 1) * TB)
                    if tb == 0:
                        xh_t, xl_t = x0
                    else:
                        xh_t, xl_t = load_x(tb, tsl)

                    n_inst = 3 * KO2
                    qk_terms = ((wqkh_sb, xh_t), (wqkl_sb, xh_t),
                                (wqkh_sb, xl_t))
                    if tb == 0:
                        # term-outer: the first 64 instructions need only the
                        # first two transfers (xh, wqkh) and run while the
                        # wqkl / xl streams are still in flight
                        ps_qk = {d: p1p.tile([P, TB], F32, tag="ps1",
                                             name=f"psqk{tb}_{d}")
                                 for d in range(NQK)}
                        for ti, (wt, xt) in enumerate(qk_terms):
                            for a in range(KO2):
                                st = ti == 0 and a == 0
                                sp = ti == 2 and a == KO2 - 1
                                for d in range(NQK):
                                    nc.tensor.matmul(
                                        ps_qk[d][:],
                                        wt[:, a, :, d * P : (d + 1) * P],
                                        xt[:, a],
                                        start=st, stop=sp, perf_mode=DR,
                                    )
                        for d in range(NQK):
                            scaled_copy(qk_sb[d][:, tsl], ps_qk[d][:])
                    else:
                        # d-outer: each group finishes as soon as its own 24
                        # instructions retire, so its PSUM->SBUF copy drains
                        # while the next group computes and the slot frees in
                        # time for the v sweep
                        for d in range(NQK):
                            ps = p1p.tile([P, TB], F32, tag="ps1",
                                          name=f"psqk{tb}_{d}")
                            i_inst = 0
                            for a in range(KO2):
                                for (wt, xt) in qk_terms:
                                    nc.tensor.matmul(
                                        ps[:],
                                        wt[:, a, :, d * P : (d + 1) * P],
                                        xt[:, a],
                                        start=(i_inst == 0),
                                        stop=(i_inst == n_inst - 1),
                                        perf_mode=DR,
                                    )
                                    i_inst += 1
                            scaled_copy(qk_sb[d][:, tsl], ps[:])

                    v_terms = ((wvh_sb, xh_t), (wvh_sb, xl_t), (wvl_sb, xh_t))
                    for t4 in range(4):
                        ps = p1p.tile([P, HPC * HD], F32, tag="ps1",
                                      name=f"psv{tb}_{t4}")
                        i_inst = 0
                        for a in range(KO2):
                            for (wt, xt) in v_terms:
                                nc.tensor.matmul(
                                    ps[:],
                                    xt[:, a, :, t4 * P : (t4 + 1) * P],
                                    wt[:, a],
                                    start=(i_inst == 0),
                                    stop=(i_inst == n_inst - 1),
                                    perf_mode=DR,
                                )
                                i_inst += 1
                        scaled_copy(v_sb[:, tb * 4 + t4, :], ps[:])

                    # head-0 RoPE rides along with phase 1, one q and one k
                    # block per tb, borrowing p1 PSUM slots
                    rope_block(0, 0, tb)
                    rope_block(0, 1, tb)
                    if tb == 0:
                        # scheduler-only fence after the first t-block: stops
                        # the list scheduler from hoisting phase-2 work in
                        # front of phase 1's DMA-paced opening, while still
                        # letting it weave head-0 score blocks into the
                        # later t-blocks' streams
                        tc.no_sync_barrier()

            # ------------------------------------------------------- p2 + p3
            with (
                tc.tile_pool(name="p3w", bufs=1) as p3w,
                tc.tile_pool(name="pt", bufs=1) as pt_pool,
                tc.tile_pool(name="ts0", bufs=2) as ts0_pool,
                tc.tile_pool(name="tsr", bufs=1) as tsr_pool,
                tc.tile_pool(name="y3", bufs=3) as y3_pool,
                tc.tile_pool(name="pst", bufs=2, space="PSUM") as pst_pool,
                tc.tile_pool(name="po", bufs=2, space="PSUM") as po_pool,
                tc.tile_pool(name="pd", bufs=1, space="PSUM") as pd_pool,
                tc.tile_pool(name="pjy", bufs=1, space="PSUM") as pjy_pool,
            ):
                wo_sb = []
                for wi, src in enumerate((woh, wol)):
                    t = p3w.tile([P, 2, 2, D], FP8, tag=f"wo{wi}",
                                 name=f"wo{wi}")
                    nc.sync.dma_start(
                        t[:], src.rearrange("(a j p) n -> p a j n", j=2, p=P))
                    wo_sb.append(t)

                # probabilities for the current head, [k-part, kb, q]
                pt = pt_pool.tile([P, NKB, T], F16)

                # per-head kb-quad partial sums of pt for the denominator:
                # quad j (kbs 4j..4j+3) lives at column offset OFFJ[j] and
                # covers q in [512j, 2048).  Collapsing pt on DVE/Pool turns
                # the per-kb ones-matmul sweep (17408 rows/head) into one
                # accumulating matmul per quad (5120 rows/head).
                # quad 0 is built across the head boundary (boot) so it
                # needs 2 rotating buffers; quads 1-3 of head h+1 are only
                # built after head h's last denominator read, so one buffer
                # suffices.  T_j is only ever read at q >= 512(j+1): the
                # quad's own diagonal q-range is served per-kb from pt.
                OFFJ = [0, 0, 1024]
                ts0_tiles = []
                tsr_tiles = []

                def alloc_tsum(h):
                    ts0_tiles.append(ts0_pool.tile([P, 1536], F16, tag="ts0",
                                                   name=f"ts0_{h}"))

                def alloc_tsr(h):
                    tsr_tiles.append(tsr_pool.tile([P, 1536], F16, tag="tsr",
                                                   name=f"tsr{h}"))

                def tsq(h, j, qlo, qhi):
                    t = ts0_tiles[h] if j == 0 else tsr_tiles[h]
                    base = 512 * (j + 1)
                    return t[:, OFFJ[j] + qlo - base : OFFJ[j] + qhi - base]

                qadd_rot = [0]

                def quad_add(h, j):
                    """T_j = pt[4j]+pt[4j+1]+pt[4j+2]+pt[4j+3] over the quad's
                    read range q >= 512(j+1); below that the denominator
                    reads pt per-kb directly."""
                    lo = TB * (j + 1)
                    for r in range(1, 4):
                        kb = 4 * j + r
                        dst = tsq(h, j, lo, T)
                        src = pt[:, kb, lo:T]
                        first = pt[:, 4 * j, lo:T] if r == 1 else dst
                        nc.vector.tensor_tensor(dst, first, src,
                                                mybir.AluOpType.add)

                def st_exp_block(h, kb):
                    """Scores + exp for one k-block, fine-causal: only
                    q >= 128*kb.  Chunks pair into one 2-bank PSUM tile so
                    each exp covers up to 1024 columns."""
                    qr, kr = roped[h]
                    q0 = kb * P
                    tq0 = kb // 4
                    bounds = [q0] + [t * TB for t in range(tq0 + 1, NTB)] + [T]
                    widths = [bounds[i + 1] - bounds[i]
                              for i in range(len(bounds) - 1)]
                    ci = 0
                    while ci < len(widths):
                        take = min(2, len(widths) - ci)
                        if take == 1 and h < HPC - 1:
                            # odd trailing chunk: borrow the projection-drip
                            # bank (idle until the last head) instead of a
                            # full 2-bank pst slot -- a third slot for the
                            # score->exp pipeline
                            ps = pjy_pool.tile([P, TB], F32, tag="pjy",
                                               name=f"pstj{h}_{kb}_{ci}")
                        else:
                            ps = pst_pool.tile([P, 2 * TB], F32, tag="pst",
                                               name=f"pst{h}_{kb}_{ci}")
                        lo = bounds[ci]
                        if take == 2:
                            wa, wb = widths[ci], widths[ci + 1]
                            nc.tensor.matmul(ps[:, TB - wa : TB],
                                             kr[:, q0 : q0 + P],
                                             qr[:, lo : lo + wa],
                                             start=True, stop=True)
                            nc.tensor.matmul(ps[:, TB : TB + wb],
                                             kr[:, q0 : q0 + P],
                                             qr[:, lo + wa : lo + wa + wb],
                                             start=True, stop=True)
                            nc.scalar.activation(
                                pt[:, kb, lo : lo + wa + wb],
                                ps[:, TB - wa : TB + wb],
                                mybir.ActivationFunctionType.Exp,
                                scale=EXP_SCALE, bias=ebias_sb[:])
                        else:
                            w = widths[ci]
                            nc.tensor.matmul(ps[:, :w], kr[:, q0 : q0 + P],
                                             qr[:, lo : lo + w],
                                             start=True, stop=True)
                            nc.scalar.activation(
                                pt[:, kb, lo : lo + w], ps[:, :w],
                                mybir.ActivationFunctionType.Exp,
                                scale=EXP_SCALE, bias=ebias_sb[:])
                        ci += take
                    # causal mask on the diagonal 128 columns
                    nc.vector.tensor_tensor(
                        pt[:, kb, q0 : q0 + P], pt[:, kb, q0 : q0 + P],
                        maskd_sb[:], mybir.AluOpType.mult)

                def pv_block(h, tq, boot=None):
                    """PV + denominator + normalize + fp8 hi/lo split for one
                    512-wide q-block.  When `boot` is given (last q-block of a
                    head), the next head's first score blocks are interleaved
                    into the accumulation stream right after their pt region's
                    final read, hiding the head-boundary serialization."""
                    tsl = slice(tq * TB, (tq + 1) * TB)
                    ps_o = po_pool.tile([P, TB], F32, tag="po",
                                        name=f"po{h}{tq}")
                    ps_d = pd_pool.tile([P, TB], F32, tag="pd",
                                        name=f"pd{h}{tq}")
                    nkb = 4 * tq + 4
                    for kb in range(nkb):
                        lo = max(tq * TB, kb * P)
                        osl = slice(lo - tq * TB, TB)
                        psl = slice(lo, (tq + 1) * TB)
                        st, sp = (kb == 0), (kb == nkb - 1)
                        nc.tensor.matmul(
                            ps_o[:, osl], v_sb[:, kb, h * HD : (h + 1) * HD],
                            pt[:, kb, psl], start=st, stop=sp)
                        if boot and kb in (1, 3, 5, 7, 9, 11, 13, 15):
                            st_exp_block(h + 1, boot.pop(0))
                    # denominator: one accumulating ones-matmul per COMPLETED
                    # kb-quad (its adds ran a full q-block ago, off the
                    # critical path); the diagonal quad uses two DVE pair-sums
                    # so it costs 768 matmul rows instead of 1280.
                    q0 = tq * TB
                    kb0 = 4 * tq
                    pd2 = t16_pool.tile([P, 2, TB], F16, tag="pd2",
                                        name=f"pd2_{h}{tq}")
                    nc.vector.tensor_tensor(
                        pd2[:, 0, P:TB], pt[:, kb0, q0 + P : q0 + TB],
                        pt[:, kb0 + 1, q0 + P : q0 + TB], mybir.AluOpType.add)
                    nc.vector.tensor_tensor(
                        pd2[:, 1, 3 * P : TB],
                        pt[:, kb0 + 2, q0 + 3 * P : q0 + TB],
                        pt[:, kb0 + 3, q0 + 3 * P : q0 + TB],
                        mybir.AluOpType.add)
                    for j in range(tq):
                        nc.tensor.matmul(ps_d[:, :], ones_sb[:],
                                         tsq(h, j, q0, q0 + TB),
                                         start=(j == 0), stop=False)
                    st0 = tq == 0
                    nc.tensor.matmul(ps_d[:, :P], ones_sb[:],
                                     pt[:, kb0, q0 : q0 + P],
                                     start=st0, stop=False)
                    nc.tensor.matmul(ps_d[:, P:TB], ones_sb[:],
                                     pd2[:, 0, P:TB], start=st0, stop=False)
                    nc.tensor.matmul(ps_d[:, 2 * P : 3 * P], ones_sb[:],
                                     pt[:, kb0 + 2, q0 + 2 * P : q0 + 3 * P],
                                     start=False, stop=False)
                    nc.tensor.matmul(ps_d[:, 3 * P : TB], ones_sb[:],
                                     pd2[:, 1, 3 * P : TB],
                                     start=False, stop=True)
                    # normalize + fp8 split feed only phase 3 (a full head
                    # later for h<3), so the whole chain rides the otherwise
                    # idle Q7 Pool engine, keeping DVE free for rope/mask/adds
                    rec = rec_pool.tile([P, TB], F32, tag="rec",
                                        name=f"rec{h}{tq}")
                    nc.vector.reciprocal(rec[:], ps_d[:])
                    t16 = t16_pool.tile([P, TB], F16, tag="t16",
                                        name=f"t16{h}{tq}")
                    nc.vector.tensor_tensor(t16[:], ps_o[:], rec[:],
                                            mybir.AluOpType.mult)
                    a, j = h // 2, h % 2
                    nc.gpsimd.tensor_copy(o_hi[a][:, j, tsl], t16[:])
                    nc.gpsimd.tensor_tensor(o_lo[a][:, j, tsl], t16[:],
                                            o_hi[a][:, j, tsl],
                                            mybir.AluOpType.subtract)

                y_tiles = {}

                def p3_group(tt, dd, pool, ptag, pshape, drip=False):
                    """One [128t x 512D] output-projection PSUM group.  The
                    four dd chunks of a t-tile gather in one SBUF tile and
                    ship as a single store on the otherwise-idle Pool SWDGE
                    queue."""
                    tq = tt // 4
                    off = (tt % 4) * P
                    tslq = slice(tq * TB + off, tq * TB + off + P)
                    psy = pool.tile(pshape, F32, tag=ptag, name=f"py{tt}_{dd}")
                    dsl = slice(dd * TB, (dd + 1) * TB)
                    i_inst = 0
                    for a in range(2):
                        for (ot, wt) in ((o_hi[a], wo_sb[0]),
                                         (o_lo[a], wo_sb[0]),
                                         (o_hi[a], wo_sb[1])):
                            nc.tensor.matmul(
                                psy[:, :TB], ot[:, :, tslq],
                                wt[:, a, :, dsl],
                                start=(i_inst == 0), stop=(i_inst == 5),
                                perf_mode=DR)
                            i_inst += 1
                    if dd == 0:
                        y_tiles[tt] = y3_pool.tile([P, D // TB, TB], F16,
                                                   tag="ysb", name=f"ysb{tt}")
                    ysb = y_tiles[tt]
                    if tt >= 12:
                        # kernel tail: alternate so the final chunk's copy
                        # rides DVE (Activation drains the previous one)
                        (nc.scalar.copy if dd % 2 == 0
                         else nc.vector.tensor_copy)(ysb[:, dd, :],
                                                     psy[:, :TB])
                        # per-dd chunk stores so the final transfer is 128 KB;
                        # the very last chunk takes the idle software DGE,
                        # bypassing the HWDGE queue backlog
                        if tt == 15 and dd == 3:
                            q = nc.gpsimd
                        else:
                            q = nc.sync if (tt + dd) % 2 == 0 else nc.scalar
                        q.dma_start(
                            y[tt * P : (tt + 1) * P, dd * TB : (dd + 1) * TB],
                            ysb[:, dd, :])
                        if dd == D // TB - 1:
                            y_tiles.pop(tt)
                    else:
                        (nc.vector.tensor_copy if dd % 2 == 0
                         else nc.scalar.copy)(ysb[:, dd, :], psy[:, :TB])
                        if dd == D // TB - 1:
                            (nc.scalar.dma_start if tt % 2 == 0
                             else nc.sync.dma_start)(
                                y[tt * P : (tt + 1) * P, :],
                                y_tiles.pop(tt)[:])

                # drips alternate over the pjy bank and the pd bank (pd is
                # only held during a PV accumulation group, which never
                # overlaps the drip points)
                p3_pending = []
                p3_pools = [(pjy_pool, "pjy", [P, TB]), (pd_pool, "pd", [P, TB])]
                p3_rot = [0]

                def p3_drip(n):
                    for _ in range(min(n, len(p3_pending))):
                        tt, dd = p3_pending.pop(0)
                        pool, ptag, pshape = p3_pools[p3_rot[0] % len(p3_pools)]
                        p3_rot[0] += 1
                        p3_group(tt, dd, pool, ptag, pshape, drip=True)

                boot_done = set()
                rope_q = []

                def la_slot(h, kb):
                    if kb < NKB and (h, kb) not in boot_done:
                        st_exp_block(h, kb)
                    if rope_q:
                        rope_block(*rope_q.pop(0))
                    if rope_q:
                        rope_block(*rope_q.pop(0))
                    if rope_q:
                        rope_block(*rope_q.pop(0))
                    p3_drip(3)

                for h in range(HPC):
                    if h + 1 < HPC:
                        load_cs(h + 1)
                        roped.append(alloc_rope())
                    # next head's rope blocks, drip-fed between score blocks
                    rope_q.clear()
                    rope_q.extend([(h + 1, w, tb) for w in (0, 1)
                                   for tb in range(NTB)]
                                  if h + 1 < HPC else [])
                    # quads 0-1 score+exp (boot pre-computed them for h>0)
                    for kb in range(8):
                        if (h, kb) not in boot_done:
                            st_exp_block(h, kb)
                    for tq in range(NTB):
                        boot = None
                        if tq == NTB - 1 and h + 1 < HPC:
                            boot = list(range(8))
                            boot_done.update((h + 1, kb) for kb in boot)
                        pv_block(h, tq, boot=boot)
                        if tq == 0:
                            alloc_tsum(h)
                            alloc_tsr(h)
                        if tq <= 2:
                            quad_add(h, tq)
                        if h == HPC - 1 and tq > 0:
                            # previous q-block's projection is now unblocked;
                            # drip its 16 groups between attention work so the
                            # single PSUM slot's copy latency stays hidden
                            p3_pending.extend(
                                (tt, dd) for tt in range((tq - 1) * 4, tq * 4)
                                for dd in range(D // TB))
                        p3_drip(6)
                        # lookahead: quad tq+2 scores land a full q-block
                        # ahead of their first PV/denominator read; rope and
                        # projection drips ride the same slots
                        for i in range(4):
                            la_slot(h, 4 * (tq + 2) + i)
                        if h == HPC - 1 and tq == 1:
                            # last score block emitted: pst slots free up,
                            # let the projection drip rotate over them
                            p3_pools.append((pst_pool, "pst", [P, 2 * TB]))

                # tail: remaining projection groups rotate over the now-idle
                # attention PSUM pools
                tail = (list(p3_pending)
                        + [(tt, dd) for tt in range(12, 16)
                           for dd in range(D // TB)])
                p3_pending.clear()
                pools3 = [(pjy_pool, "pjy", [P, TB]), (po_pool, "po", [P, TB]),
                          (pd_pool, "pd", [P, TB]),
                          (pst_pool, "pst", [P, 2 * TB])]
                for i, (tt, dd) in enumerate(tail):
                    p3_group(tt, dd, *pools3[i % len(pools3)])

                if debug:
                    for d in range(NQK):
                        nc.sync.dma_start(dqk[d], qk_sb[d][:])
                    nc.sync.dma_start(dv[:], v_sb[:])
                    nc.sync.dma_start(dpt[:], pt[:])  # head 3's state
                    for a2 in range(2):
                        nc.sync.dma_start(doh[a2], o_hi[a2][:])
                        nc.sync.dma_start(dol[a2], o_lo[a2][:])
                    nc.sync.dma_start(drp[0], roped[HPC - 1][0][:])
                    nc.sync.dma_start(drp[1], roped[HPC - 1][1][:])

    _fix_waits(nc)
    return nc


_NC_CACHE = None


def _get_program():
    global _NC_CACHE
    if _NC_CACHE is None:
        _NC_CACHE = _build_program()
    return _NC_CACHE


def _host_inputs(x, Wqkv, Wout, cos, sin, rope_mask):
    """Build the 8 per-core input maps (fp8 hi/lo splits on the host)."""
    import ml_dtypes

    E4 = ml_dtypes.float8_e4m3
    x = np.asarray(x, dtype=np.float32)
    Wqkv = np.asarray(Wqkv, dtype=np.float32)
    Wout = np.asarray(Wout, dtype=np.float32)
    cos = np.asarray(cos, dtype=np.float32)
    sin = np.asarray(sin, dtype=np.float32)
    rope_mask = np.asarray(rope_mask).astype(bool)

    def split8(a):
        hi = a.astype(E4)
        lo = (a - hi.astype(np.float32)).astype(E4)
        return np.ascontiguousarray(hi), np.ascontiguousarray(lo)

    maskd = (np.arange(P)[:, None] <= np.arange(P)[None, :]).astype(np.float16)
    onesv = np.full((P, P), QS / SO, dtype=np.float16)

    C_full = np.repeat(cos[:T].T, 2, axis=0).astype(np.float16)  # [128, T]
    S_full = np.repeat(sin[:T].T, 2, axis=0).astype(np.float16)
    S_full[0::2] *= np.float16(-1)  # rotation signs folded into the table
    C_id = np.ones_like(C_full)
    S_id = np.zeros_like(S_full)

    xs = [split8(x[b].T) for b in range(B)]

    in_maps = []
    for c in range(N_CORES):
        b = c // CORES_PER_B
        hg = c % CORES_PER_B
        heads = [hg * HPC + i for i in range(HPC)]

        qrows = np.concatenate([np.arange(h * HD, (h + 1) * HD) for h in heads])
        krows = qrows + D
        vrows = qrows + 2 * D
        wqkh_a, wqkl_a = split8(SW * Wqkv[np.concatenate([qrows, krows])].T)
        wvh_a, wvl_a = split8(SW * Wqkv[vrows].T)
        woh_a, wol_a = split8(SW * Wout[:, qrows].T)

        cs_arr = np.empty((HPC, 2, P, T), dtype=np.float16)
        for i, h in enumerate(heads):
            cs_arr[i, 0] = C_full if rope_mask[h] else C_id
            cs_arr[i, 1] = S_full if rope_mask[h] else S_id

        in_maps.append(
            {
                "xh8": xs[b][0],
                "xl8": xs[b][1],
                "wqkh": wqkh_a,
                "wqkl": wqkl_a,
                "wvh": wvh_a,
                "wvl": wvl_a,
                "woh": woh_a,
                "wol": wol_a,
                "ones": onesv,
                "cs": cs_arr,
                "maskd": maskd,
                "ebias": np.full((P, 1), EXP_BIAS, dtype=np.float32),
            }
        )
    return in_maps


def kernel(x, Wqkv, Wout, cos, sin, rope_mask, _trace=False):
    nc = _get_program()
    in_maps = _host_inputs(x, Wqkv, Wout, cos, sin, rope_mask)
    res = run_bass_kernel_spmd(nc, in_maps, core_ids=list(range(N_CORES)),
                               trace=_trace)
    inv = 1.0 / (SO * SW)
    parts = [res.results[c]["y"].astype(np.float32) for c in range(N_CORES)]
    out = np.stack(
        [inv * sum(parts[b * CORES_PER_B : (b + 1) * CORES_PER_B])
         for b in range(B)]
    ).astype(np.float32)
    if _trace:
        kernel.last_result = res
    return out

